# revision 35
# baseline (speedup 1.0000x reference)
"""Causal multi-head attention block on 8 Trainium2 NeuronCores.

Problem: x:[2,2048,1024] f32 -> MHA(H=16 heads, dk=dv=64, causal) -> [2,2048,1024].

Distribution (tensor-parallel heads, row-parallel output projection):
  - Each core c owns heads {2c, 2c+1}: it gets the matching 128-column slices
    of Wq/Wk/Wv and the matching 128-ROW slice of Wo.
  - Each core computes causal attention for its two heads over all 4096
    (batch*seq) rows, then the row-parallel partial out_c = A_c @ Wo_c for
    ALL rows.  The host sums the 8 partials (the unshard for row-parallel
    TP) and adds bo.  No device collective at all.

Compute dtype bf16 (fp32 PSUM accumulation).  Host supplies x^T pre-cast to
bf16.  Softmax skips the running-max (logits ~N(0,1); exp cannot overflow)
and gets its denominator for free from ones-columns appended to V.
Everything except exp runs off the scalar engine so ACT does softmax only;
projections for later strips and the previous strip's output projection are
interleaved into the attention k-tile loop to keep the PE warm.
"""

import numpy as np
import ml_dtypes

import concourse.mybir as mybir
from concourse import bacc
from concourse.bass_utils import run_bass_kernel_spmd
from concourse.tile import TileContext

F32 = mybir.dt.float32
BF16 = mybir.dt.bfloat16
BF16_NP = ml_dtypes.bfloat16

B, S, D = 2, 2048, 1024
H, DK, DV = 16, 64, 64
ROWS = B * S                  # 4096
NCORES = 8
HPC = H // NCORES             # 2 heads per core
HD = HPC * DK                 # 128 per-core head dim
NSTRIP = ROWS // 512          # 8 global 512-row strips
SCALE = 1.0 / np.sqrt(DK)

# attention strip order: heavy (late, causal) strips first so projections
# pipeline into their tails;  (b, s) -> strip g = b*4 + s
ATTN_ORDER = [(0, 3), (0, 2), (1, 3), (0, 1), (1, 2), (0, 0), (1, 1), (1, 0)]


def _build():
    nc = bacc.Bacc(None, target_bir_lowering=False, debug=False)

    xT = nc.declare_dram_parameter("xT", [D, ROWS], BF16, isOutput=False)
    wq = nc.declare_dram_parameter("wq", [D, HD], BF16, isOutput=False)
    wk = nc.declare_dram_parameter("wk", [D, HD], BF16, isOutput=False)
    wv = nc.declare_dram_parameter("wv", [D, HD], BF16, isOutput=False)
    bq = nc.declare_dram_parameter("bq", [HD, 1], F32, isOutput=False)
    bk = nc.declare_dram_parameter("bk", [HD, 1], F32, isOutput=False)
    bv = nc.declare_dram_parameter("bv", [1, HD], F32, isOutput=False)
    wo = nc.declare_dram_parameter("wo", [HD, D], BF16, isOutput=False)
    # per-head partial outputs (normalization is folded into the PSUM
    # evacuation per head; the host sums 2 x 8 partials)
    out0 = nc.declare_dram_parameter("out0", [ROWS, D], BF16, isOutput=True)
    out1 = nc.declare_dram_parameter("out1", [ROWS, D], BF16, isOutput=True)

    with TileContext(nc) as tc:
        with tc.tile_pool(name="const", bufs=1) as csb, \
             tc.tile_pool(name="dram", bufs=1, space="DRAM") as dpool, \
             tc.tile_pool(name="sc_ps", bufs=2, space="PSUM") as sc_ps, \
             tc.tile_pool(name="pv_ps", bufs=2, space="PSUM") as pv_ps, \
             tc.tile_pool(name="mm_ps", bufs=2, space="PSUM") as mm_ps, \
             tc.tile_pool(name="es_sb", bufs=4) as es_sb, \
             tc.tile_pool(name="at_sb", bufs=3) as at_sb, \
             tc.tile_pool(name="den_sb", bufs=6) as den_sb, \
             tc.tile_pool(name="osb", bufs=6) as osb_pool:

            # ---------------- constants / weights ----------------
            # triangle keep-mask: mask[kr, q] = 1 if kr <= q else 0
            trimask = csb.tile([128, 128], BF16, name="trimask")
            nc.gpsimd.memset(trimask[:], 1.0)
            nc.gpsimd.affine_select(
                out=trimask[:], in_=trimask[:],
                compare_op=mybir.AluOpType.is_ge, fill=0.0,
                base=0, pattern=[[1, 128]], channel_multiplier=-1,
            )

            wq_sb = csb.tile([128, D], BF16, name="wq_sb")
            wk_sb = csb.tile([128, D], BF16, name="wk_sb")
            wv_sb = csb.tile([128, D], BF16, name="wv_sb")
            wo_sb = csb.tile([128, D], BF16, name="wo_sb")
            bq_sb = csb.tile([HD, 1], F32, name="bq_sb")
            bk_sb = csb.tile([HD, 1], F32, name="bk_sb")
            bv_bc = csb.tile([128, HD], F32, name="bv_bc")
            nc.sync.dma_start(out=wq_sb[:].rearrange("p (a c) -> p a c", a=8), in_=wq[:].rearrange("(a p) c -> p a c", p=128))
            nc.sync.dma_start(out=wk_sb[:].rearrange("p (a c) -> p a c", a=8), in_=wk[:].rearrange("(a p) c -> p a c", p=128))
            nc.sync.dma_start(out=wv_sb[:].rearrange("p (a c) -> p a c", a=8), in_=wv[:].rearrange("(a p) c -> p a c", p=128))
            nc.sync.dma_start(out=wo_sb[:], in_=wo[:])
            nc.sync.dma_start(out=bq_sb[:], in_=bq[:])
            nc.sync.dma_start(out=bk_sb[:], in_=bk[:])
            nc.sync.dma_start(out=bv_bc[:], in_=bv[:].to_broadcast([128, HD]))

            # x^T resident in SBUF: per d-block, 2 half tiles [128, 2048].
            # Half 0 (strips 0-3) loads now; half 1 is emitted later (first
            # needed by the strip-4..7 projections that drip into attention).
            xt_sb = [[csb.tile([128, 2048], BF16, name=f"xt{d}_{h}")
                      for h in range(2)] for d in range(8)]
            for d in range(8):
                nc.sync.dma_start(
                    out=xt_sb[d][0][:], in_=xT[d * 128:(d + 1) * 128, 0:2048])

            def load_xt_half1():
                for d in range(8):
                    nc.scalar.dma_start(
                        out=xt_sb[d][1][:], in_=xT[d * 128:(d + 1) * 128, 2048:4096])

            def xts(d, g, lo, width):
                # x^T slice [128, width] for strip g starting at column lo
                h, off = divmod(g, 4)
                return xt_sb[d][h][:, off * 512 + lo: off * 512 + lo + width]

            # PE clock warm-up while the x DMAs land
            warm = csb.tile([128, 512], BF16, name="warm")
            nc.gpsimd.memset(warm[:], 0.0)
            wps = mm_ps.tile([128, 512], F32, tag="mm", name="warm_ps")
            for i in range(28):
                nc.tensor.matmul(wps[:], lhsT=warm[:, 0:128], rhs=warm[:],
                                 start=(i == 0), stop=(i == 27))

            den_dram = dpool.tile([2 * NSTRIP, 512], F32, name="den_dram")

            # long-lived per-strip tensors
            qT = [csb.tile([128, 512], BF16, name=f"qT{g}") for g in range(NSTRIP)]
            kTt = [csb.tile([128, 512], BF16, name=f"kT{g}") for g in range(NSTRIP)]
            # v_strip[g]: 4 k-tiles x [v0(64) | one | one | v1(64)] = [128, 520]
            v_strip = [csb.tile([128, 4 * 130], BF16, name=f"v{g}") for g in range(NSTRIP)]

            # ---------------- building blocks ----------------
            def proj_qk(g, w_sb, b_sb, dst):
                ps = mm_ps.tile([128, 512], F32, tag="mm", name=f"qk_ps_{g}")
                for d in range(8):
                    nc.tensor.matmul(
                        ps[:], lhsT=w_sb[:, d * 128:(d + 1) * 128],
                        rhs=xts(d, g, 0, 512), start=(d == 0), stop=(d == 7))
                nc.vector.tensor_scalar_add(dst[g][:], ps[:], b_sb[:])

            def proj_v(g):
                # V in [rows, dv] layout: 4 row-blocks of [128, 128] in one tile
                ps = mm_ps.tile([128, 512], F32, tag="mm", name=f"v_ps_{g}")
                for rb in range(4):
                    for d in range(8):
                        nc.tensor.matmul(
                            ps[:, rb * 128:(rb + 1) * 128],
                            lhsT=xts(d, g, rb * 128, 128),
                            rhs=wv_sb[:, d * 128:(d + 1) * 128],
                            start=(d == 0), stop=(d == 7))
                nc.gpsimd.memset(v_strip[g][:], 1.0)
                ps3 = ps[:].rearrange("p (k c) -> p k c", k=4)
                v3 = v_strip[g][:].rearrange("p (k c) -> p k c", k=4)
                bv3_0 = bv_bc[:, 0:64].unsqueeze(1).to_broadcast([128, 4, 64])
                bv3_1 = bv_bc[:, 64:128].unsqueeze(1).to_broadcast([128, 4, 64])
                nc.vector.tensor_tensor(
                    v3[:, :, 0:64], ps3[:, :, 0:64], bv3_0, mybir.AluOpType.add)
                nc.vector.tensor_tensor(
                    v3[:, :, 65:129], ps3[:, :, 64:128], bv3_1, mybir.AluOpType.add)

            ready = set()

            def chunk_q(g):
                return lambda: (ready.add(("q", g)), proj_qk(g, wq_sb, bq_sb, qT))

            def chunk_k(g):
                return lambda: (ready.add(("k", g)), proj_qk(g, wk_sb, bk_sb, kTt))

            def chunk_v(g):
                return lambda: (ready.add(("v", g)), proj_v(g))

            # state for the strip whose epilogue/out-proj is still pending
            pending_oproj = []

            def emit_oproj(g, at, dent):
                # outh[g*512 + qb*128 .. , :] = (at_h[:, qb].T @ Wo_h) / den_h
                # Per-head matmuls run concurrently as PE row-tiles (0,0) and
                # (64,0) into different PSUM banks; the 1/den softmax scale is
                # a per-PARTITION scalar at evacuation (output rows = q), so
                # the den chain is entirely off the PE critical path.
                for qb in range(4):
                    ot0 = osb_pool.tile([128, D], BF16, tag="ot", name=f"ot0_{g}_{qb}")
                    ot1 = osb_pool.tile([128, D], BF16, tag="ot", name=f"ot1_{g}_{qb}")
                    for n in range(2):
                        ps0 = mm_ps.tile([128, 512], F32, tag="mm", name=f"o_ps0_{g}_{qb}_{n}")
                        ps1 = mm_ps.tile([128, 512], F32, tag="mm", name=f"o_ps1_{g}_{qb}_{n}")
                        nc.tensor.matmul(
                            ps0[:], lhsT=at[0:64, qb * 128:(qb + 1) * 128],
                            rhs=wo_sb[0:64, n * 512:(n + 1) * 512],
                            start=True, stop=True)
                        nc.tensor.matmul(
                            ps1[:], lhsT=at[64:128, qb * 128:(qb + 1) * 128],
                            rhs=wo_sb[64:128, n * 512:(n + 1) * 512],
                            start=True, stop=True)
                        nc.scalar.activation(
                            ot0[:, n * 512:(n + 1) * 512], ps0[:],
                            mybir.ActivationFunctionType.Identity,
                            scale=dent[:, qb:qb + 1])
                        nc.vector.tensor_scalar_mul(
                            ot1[:, n * 512:(n + 1) * 512], ps1[:],
                            dent[:, 4 + qb:5 + qb])
                    r = slice(g * 512 + qb * 128, g * 512 + (qb + 1) * 128)
                    nc.sync.dma_start(out=out0[r, :], in_=ot0[:])
                    nc.sync.dma_start(out=out1[r, :], in_=ot1[:])

            def attn_strip(b, s, bg):
                g = b * 4 + s
                njt = 4 * (s + 1)
                assert ("q", g) in ready, f"qT[{g}] not emitted"
                pv0 = pv_ps.tile([65, 512], F32, tag="pv", name=f"pv0_{g}")
                pv1 = pv_ps.tile([65, 512], F32, tag="pv", name=f"pv1_{g}")
                for j in range(njt):
                    gk = b * 4 + j // 4
                    jj = j % 4
                    qlo = max(0, j - 4 * s) * 128
                    assert ("k", gk) in ready, f"kT[{gk}] not emitted (strip {g} j={j})"
                    assert ("v", gk) in ready, f"v[{gk}] not emitted (strip {g} j={j})"
                    sc = sc_ps.tile([128, 1024], F32, tag="sc", name=f"sc_{g}_{j}")
                    nc.tensor.matmul(
                        sc[:, qlo:512],
                        lhsT=kTt[gk][0:64, jj * 128:(jj + 1) * 128],
                        rhs=qT[g][0:64, qlo:512], start=True, stop=True)
                    nc.tensor.matmul(
                        sc[:, 512 + qlo:1024],
                        lhsT=kTt[gk][64:128, jj * 128:(jj + 1) * 128],
                        rhs=qT[g][64:128, qlo:512], start=True, stop=True)
                    es = es_sb.tile([128, 1024], BF16, tag="es", name=f"es_{g}_{j}")
                    nc.scalar.activation(
                        es[:, qlo:1024], sc[:, qlo:1024],
                        mybir.ActivationFunctionType.Exp, scale=SCALE)
                    if j >= 4 * s:  # diagonal k-tile: zero kr > q inside the block
                        es3 = es[:].rearrange("p (h w) -> p h w", h=2)[:, :, qlo:qlo + 128]
                        m3 = trimask[:].unsqueeze(1).to_broadcast([128, 2, 128])
                        nc.vector.tensor_tensor(es3, es3, m3, mybir.AluOpType.mult)
                    vb = v_strip[gk][:, jj * 130: (jj + 1) * 130]
                    nc.tensor.matmul(
                        pv0[:, qlo:512], lhsT=vb[:, 0:65],
                        rhs=es[:, qlo:512],
                        start=(j == 0), stop=(j == njt - 1))
                    nc.tensor.matmul(
                        pv1[:, qlo:512], lhsT=vb[:, 65:130],
                        rhs=es[:, 512 + qlo:1024],
                        start=(j == 0), stop=(j == njt - 1))
                    # fill PE bubbles: previous strip's out-proj once its den
                    # chain has had time to land, then background projections
                    if j == 7 and pending_oproj:
                        emit_oproj(*pending_oproj.pop(0))
                    if j % 3 == 2 and bg:
                        bg.pop(0)()
                # epilogue: evacuate raw PV to bf16 `at` (frees the pv banks
                # in ~1us; head1 takes the DVE crossbar partition shift), and
                # kick off the denominator chain: den rows -> DRAM -> [128,8]
                # reciprocal, laid out so dent[p, h*4+qb] = 1/den_h[qb*128+p]
                # (the per-partition evacuation scalar for the out-proj).
                at = at_sb.tile([128, 512], BF16, tag="at", name=f"at_{g}")
                nc.vector.tensor_copy(at[0:64, :], pv0[0:64, :])
                nc.vector.tensor_copy(at[64:128, :], pv1[0:64, :])
                den0 = den_sb.tile([1, 512], F32, tag="den", name=f"den0_{g}")
                den1 = den_sb.tile([1, 512], F32, tag="den", name=f"den1_{g}")
                nc.vector.tensor_copy(den0[:], pv0[64:65, :])
                nc.vector.tensor_copy(den1[:], pv1[64:65, :])
                nc.gpsimd.dma_start(out=den_dram[2 * g], in_=den0[0:1, :])
                nc.gpsimd.dma_start(out=den_dram[2 * g + 1], in_=den1[0:1, :])
                dent = den_sb.tile([128, 8], F32, tag="dent", name=f"dent_{g}")
                nc.gpsimd.dma_start(
                    out=dent[:].rearrange("p (a b) -> p a b", a=2),
                    in_=den_dram[2 * g:2 * g + 2, :].rearrange("a (b p) -> p a b", p=128))
                nc.vector.reciprocal(dent[:], dent[:])
                pending_oproj.append((g, at, dent))

            # ---------------- schedule ----------------
            # strips 0-3's projections up front (attn(0,3) needs all of them);
            # batch-1 projections drip into the attention loop, ordered so
            # every tile is emitted before its first reader:
            #   attn(1,3) [3rd strip] needs Q7 K7 V4-V7; Q/K of 4-6 later.
            for g in (0, 1):
                chunk_q(g)(); chunk_k(g)(); chunk_v(g)()
            load_xt_half1()
            for g in (2, 3):
                chunk_q(g)(); chunk_k(g)(); chunk_v(g)()
            bg = [chunk_v(4), chunk_v(5), chunk_q(7), chunk_k(7),
                  chunk_v(6), chunk_v(7), chunk_k(4), chunk_k(5),
                  chunk_k(6), chunk_q(4), chunk_q(5), chunk_q(6)]
            for b, s in ATTN_ORDER:
                attn_strip(b, s, bg)
            while bg:
                bg.pop(0)()
            while pending_oproj:
                emit_oproj(*pending_oproj.pop(0))

    nc.finalize()
    return nc


_NC = None


def _get_nc():
    global _NC
    if _NC is None:
        _NC = _build()
    return _NC


def _make_in_maps(x, Wq, bq, Wk, bk, Wv, bv, Wo, bo):
    xT = np.ascontiguousarray(x.reshape(ROWS, D).T).astype(BF16_NP)
    in_maps = []
    for c in range(NCORES):
        sl = slice(c * HD, (c + 1) * HD)
        in_maps.append({
            "xT": xT,
            "wq": np.ascontiguousarray(Wq[:, sl]).astype(BF16_NP),
            "wk": np.ascontiguousarray(Wk[:, sl]).astype(BF16_NP),
            "wv": np.ascontiguousarray(Wv[:, sl]).astype(BF16_NP),
            "bq": np.ascontiguousarray(bq[sl]).reshape(HD, 1).astype(np.float32),
            "bk": np.ascontiguousarray(bk[sl]).reshape(HD, 1).astype(np.float32),
            "bv": np.ascontiguousarray(bv[sl]).reshape(1, HD).astype(np.float32),
            "wo": np.ascontiguousarray(Wo[sl, :]).astype(BF16_NP),
        })
    return in_maps


def _run(inputs, trace=False):
    nc = _get_nc()
    ins = {k: np.asarray(v) for k, v in inputs.items()}
    in_maps = _make_in_maps(**ins)
    res = run_bass_kernel_spmd(nc, in_maps, core_ids=list(range(NCORES)), trace=trace)
    acc = np.zeros((ROWS, D), dtype=np.float32)
    for c in range(NCORES):
        acc += res.results[c]["out0"].astype(np.float32)
        acc += res.results[c]["out1"].astype(np.float32)
    acc += ins["bo"].astype(np.float32)
    return acc.reshape(B, S, D), res


def kernel(**inputs):
    out, _ = _run(inputs, trace=False)
    return out


# revision 36
# speedup vs baseline: 1.2519x; 1.2519x over previous
"""Causal multi-head attention block on 8 Trainium2 NeuronCores.

Problem: x:[2,2048,1024] f32 -> MHA(H=16 heads, dk=dv=64, causal) -> [2,2048,1024].

Distribution (tensor-parallel heads, row-parallel output projection):
  - Each core c owns heads {2c, 2c+1}: it gets the matching 128-column slices
    of Wq/Wk/Wv and the matching 128-ROW slice of Wo.
  - Each core computes causal attention for its two heads over all 4096
    (batch*seq) rows, then the row-parallel partial out_c = A_c @ Wo_c for
    ALL rows.  The host sums the 8 partials (the unshard for row-parallel
    TP) and adds bo.  No device collective at all.

Compute dtype bf16 (fp32 PSUM accumulation).  Host supplies x^T pre-cast to
bf16.  Softmax skips the running-max (logits ~N(0,1); exp cannot overflow)
and gets its denominator for free from ones-columns appended to V.

Schedule: a projection/attention ladder — proj(strip g) then attention
(strip g) — so attention (and with it ScalarE's exp stream) starts ~15us in
rather than after all projections.  Each strip's output projection is
emitted two k-tiles into the NEXT strip so its normalize chain (reciprocal
spread over 128 partitions via a DRAM bounce, broadcast back over the idle
gpsimd DMA queue) has landed by the time the PE reaches it.  Everything
except exp runs off ScalarE; PSUM evacuations are split between ScalarE
and VectorE.
"""

import numpy as np
import ml_dtypes

import concourse.mybir as mybir
from concourse import bacc
from concourse.bass_utils import run_bass_kernel_spmd
from concourse.tile import TileContext

F32 = mybir.dt.float32
BF16 = mybir.dt.bfloat16
BF16_NP = ml_dtypes.bfloat16

B, S, D = 2, 2048, 1024
H, DK, DV = 16, 64, 64
ROWS = B * S                  # 4096
NCORES = 8
HPC = H // NCORES             # 2 heads per core
HD = HPC * DK                 # 128 per-core head dim
NSTRIP = ROWS // 512          # 8 global 512-row strips
SCALE = 1.0 / np.sqrt(DK)


def _build():
    nc = bacc.Bacc(None, target_bir_lowering=False, debug=False)

    xT = nc.declare_dram_parameter("xT", [D, ROWS], BF16, isOutput=False)
    wq = nc.declare_dram_parameter("wq", [D, HD], BF16, isOutput=False)
    wk = nc.declare_dram_parameter("wk", [D, HD], BF16, isOutput=False)
    wv = nc.declare_dram_parameter("wv", [D, HD], BF16, isOutput=False)
    bq = nc.declare_dram_parameter("bq", [HD, 1], F32, isOutput=False)
    bk = nc.declare_dram_parameter("bk", [HD, 1], F32, isOutput=False)
    bv = nc.declare_dram_parameter("bv", [1, HD], F32, isOutput=False)
    wo = nc.declare_dram_parameter("wo", [HD, D], BF16, isOutput=False)
    out = nc.declare_dram_parameter("out", [ROWS, D], BF16, isOutput=True)

    with TileContext(nc) as tc:
        with tc.tile_pool(name="const", bufs=1) as csb, \
             tc.tile_pool(name="dram", bufs=1, space="DRAM") as dpool, \
             tc.tile_pool(name="sc_ps", bufs=2, space="PSUM") as sc_ps, \
             tc.tile_pool(name="pv_ps", bufs=2, space="PSUM") as pv_ps, \
             tc.tile_pool(name="mm_ps", bufs=2, space="PSUM") as mm_ps, \
             tc.tile_pool(name="es_sb", bufs=4) as es_sb, \
             tc.tile_pool(name="at_sb", bufs=3) as at_sb, \
             tc.tile_pool(name="den_sb", bufs=6) as den_sb, \
             tc.tile_pool(name="pvc_sb", bufs=6) as pvc_sb, \
             tc.tile_pool(name="osb", bufs=6) as osb_pool:

            # ---------------- constants / weights ----------------
            # triangle keep-mask: mask[kr, q] = 1 if kr <= q else 0
            trimask = csb.tile([128, 128], BF16, name="trimask")
            nc.gpsimd.memset(trimask[:], 1.0)
            nc.gpsimd.affine_select(
                out=trimask[:], in_=trimask[:],
                compare_op=mybir.AluOpType.is_ge, fill=0.0,
                base=0, pattern=[[1, 128]], channel_multiplier=-1,
            )

            wq_sb = csb.tile([128, D], BF16, name="wq_sb")
            wk_sb = csb.tile([128, D], BF16, name="wk_sb")
            wv_sb = csb.tile([128, D], BF16, name="wv_sb")
            wo_sb = csb.tile([128, D], BF16, name="wo_sb")
            bq_sb = csb.tile([HD, 1], F32, name="bq_sb")
            bk_sb = csb.tile([HD, 1], F32, name="bk_sb")
            bv_bc = csb.tile([128, HD], F32, name="bv_bc")
            nc.sync.dma_start(out=wq_sb[:].rearrange("p (a c) -> p a c", a=8), in_=wq[:].rearrange("(a p) c -> p a c", p=128))
            nc.sync.dma_start(out=bq_sb[:], in_=bq[:])
            nc.scalar.dma_start(out=wk_sb[:].rearrange("p (a c) -> p a c", a=8), in_=wk[:].rearrange("(a p) c -> p a c", p=128))
            nc.scalar.dma_start(out=wv_sb[:].rearrange("p (a c) -> p a c", a=8), in_=wv[:].rearrange("(a p) c -> p a c", p=128))
            nc.scalar.dma_start(out=bk_sb[:], in_=bk[:])
            nc.scalar.dma_start(out=bv_bc[:], in_=bv[:].to_broadcast([128, HD]))
            nc.scalar.dma_start(out=wo_sb[:], in_=wo[:])

            # x^T resident in SBUF as 4 quarter tiles [128, 1024] per d-block.
            # Only quarter 0 (strips 0-1) loads before the Tile entry barrier
            # (which gates on setup-DMA completion); the rest stream in
            # between the first projection chunks, split across both HWDGE
            # trigger queues.
            xt_sb = [[csb.tile([128, 1024], BF16, name=f"xt{d}_{q}")
                      for q in range(4)] for d in range(8)]

            def load_xt_quarter(q):
                for d in range(8):
                    eng = nc.sync if d % 2 == 0 else nc.scalar
                    eng.dma_start(
                        out=xt_sb[d][q][:],
                        in_=xT[d * 128:(d + 1) * 128, q * 1024:(q + 1) * 1024])

            load_xt_quarter(0)

            def xts(d, g, lo, width):
                # x^T slice [128, width] for strip g starting at column lo
                q, off = divmod(g, 2)
                return xt_sb[d][q][:, off * 512 + lo: off * 512 + lo + width]

            # PE clock warm-up while the setup DMAs land
            warm = csb.tile([128, 512], BF16, name="warm")
            nc.gpsimd.memset(warm[:], 0.0)
            wps = mm_ps.tile([128, 512], F32, tag="mm", name="warm_ps")
            for i in range(20):
                nc.tensor.matmul(wps[:], lhsT=warm[:, 0:128], rhs=warm[:],
                                 start=(i == 0), stop=(i == 19))

            den_dram = dpool.tile([2 * NSTRIP, 512], F32, name="den_dram")
            denr_dram = dpool.tile([2 * NSTRIP, 512], F32, name="denr_dram")

            # long-lived per-strip tensors
            qT = [csb.tile([128, 512], BF16, name=f"qT{g}") for g in range(NSTRIP)]
            kTt = [csb.tile([128, 512], BF16, name=f"kT{g}") for g in range(NSTRIP)]
            # v_strip[g]: 4 k-tiles x [v0(64) | one | v1(64) | one] = [128, 520]
            v_strip = [csb.tile([128, 4 * 130], BF16, name=f"v{g}") for g in range(NSTRIP)]

            # ---------------- building blocks ----------------
            ready = set()

            def proj_qk(g, w_sb, b_sb, dst):
                ps = mm_ps.tile([128, 512], F32, tag="mm", name=f"qk_ps_{g}")
                for d in range(8):
                    nc.tensor.matmul(
                        ps[:], lhsT=w_sb[:, d * 128:(d + 1) * 128],
                        rhs=xts(d, g, 0, 512), start=(d == 0), stop=(d == 7))
                nc.vector.tensor_scalar_add(dst[g][:], ps[:], b_sb[:])

            def proj_v(g):
                # V in [rows, dv] layout: 4 row-blocks of [128, 128] in one tile
                ps = mm_ps.tile([128, 512], F32, tag="mm", name=f"v_ps_{g}")
                for rb in range(4):
                    for d in range(8):
                        nc.tensor.matmul(
                            ps[:, rb * 128:(rb + 1) * 128],
                            lhsT=xts(d, g, rb * 128, 128),
                            rhs=wv_sb[:, d * 128:(d + 1) * 128],
                            start=(d == 0), stop=(d == 7))
                nc.gpsimd.memset(v_strip[g][:], 1.0)
                ps3 = ps[:].rearrange("p (k c) -> p k c", k=4)
                v3 = v_strip[g][:].rearrange("p (k c) -> p k c", k=4)
                bv3_0 = bv_bc[:, 0:64].unsqueeze(1).to_broadcast([128, 4, 64])
                bv3_1 = bv_bc[:, 64:128].unsqueeze(1).to_broadcast([128, 4, 64])
                nc.vector.tensor_tensor(
                    v3[:, :, 0:64], ps3[:, :, 0:64], bv3_0, mybir.AluOpType.add)
                nc.vector.tensor_tensor(
                    v3[:, :, 65:129], ps3[:, :, 64:128], bv3_1, mybir.AluOpType.add)

            def proj_strip(g):
                proj_qk(g, wq_sb, bq_sb, qT)
                proj_qk(g, wk_sb, bk_sb, kTt)
                proj_v(g)
                ready.add(g)

            pending_oproj = []

            def emit_oproj(g, at):
                # out[g*512 + qb*128 .. , :] = at[:, qb-block].T @ Wo_c
                for qb in range(4):
                    ot = osb_pool.tile([128, D], BF16, tag="ot", name=f"ot_{g}_{qb}")
                    for n in range(2):
                        ps = mm_ps.tile([128, 512], F32, tag="mm", name=f"o_ps_{g}_{qb}_{n}")
                        nc.tensor.matmul(
                            ps[:], lhsT=at[:, qb * 128:(qb + 1) * 128],
                            rhs=wo_sb[:, n * 512:(n + 1) * 512],
                            start=True, stop=True)
                        # split the PSUM->SBUF evacuations across ACT and DVE
                        if n == 0:
                            nc.scalar.copy(ot[:, n * 512:(n + 1) * 512], ps[:])
                        else:
                            nc.vector.tensor_copy(ot[:, n * 512:(n + 1) * 512], ps[:])
                    nc.sync.dma_start(
                        out=out[g * 512 + qb * 128: g * 512 + (qb + 1) * 128, :],
                        in_=ot[:])

            def attn_strip(b, s):
                g = b * 4 + s
                njt = 4 * (s + 1)
                assert g in ready, f"projections for strip {g} not emitted"
                pv0 = pv_ps.tile([65, 512], F32, tag="pv", name=f"pv0_{g}")
                pv1 = pv_ps.tile([65, 512], F32, tag="pv", name=f"pv1_{g}")
                for j in range(njt):
                    gk = b * 4 + j // 4
                    jj = j % 4
                    qlo = max(0, j - 4 * s) * 128
                    assert gk in ready, f"v/k[{gk}] not emitted (strip {g} j={j})"
                    sc = sc_ps.tile([128, 1024], F32, tag="sc", name=f"sc_{g}_{j}")
                    nc.tensor.matmul(
                        sc[:, qlo:512],
                        lhsT=kTt[gk][0:64, jj * 128:(jj + 1) * 128],
                        rhs=qT[g][0:64, qlo:512], start=True, stop=True)
                    nc.tensor.matmul(
                        sc[:, 512 + qlo:1024],
                        lhsT=kTt[gk][64:128, jj * 128:(jj + 1) * 128],
                        rhs=qT[g][64:128, qlo:512], start=True, stop=True)
                    es = es_sb.tile([128, 1024], BF16, tag="es", name=f"es_{g}_{j}")
                    nc.scalar.activation(
                        es[:, qlo:1024], sc[:, qlo:1024],
                        mybir.ActivationFunctionType.Exp, scale=SCALE)
                    if j >= 4 * s:  # diagonal k-tile: zero kr > q inside the block
                        es3 = es[:].rearrange("p (h w) -> p h w", h=2)[:, :, qlo:qlo + 128]
                        m3 = trimask[:].unsqueeze(1).to_broadcast([128, 2, 128])
                        nc.vector.tensor_tensor(es3, es3, m3, mybir.AluOpType.mult)
                    vb = v_strip[gk][:, jj * 130: (jj + 1) * 130]
                    nc.tensor.matmul(
                        pv0[:, qlo:512], lhsT=vb[:, 0:65],
                        rhs=es[:, qlo:512],
                        start=(j == 0), stop=(j == njt - 1))
                    nc.tensor.matmul(
                        pv1[:, qlo:512], lhsT=vb[:, 65:130],
                        rhs=es[:, 512 + qlo:1024],
                        start=(j == 0), stop=(j == njt - 1))
                    # previous strip's out-projection drops in once its
                    # normalize chain has had time to land
                    if j == 2 and pending_oproj:
                        emit_oproj(*pending_oproj.pop(0))
                # epilogue: evacuate PV so the pv banks free up immediately,
                # then normalize.  The reciprocal of the 2x512 denominators is
                # spread over 128 partitions via a DRAM round-trip ([1,512]
                # on one DVE lane costs ~3.3us; [128,8] costs ~70ns); all DMA
                # legs ride the otherwise-idle gpsimd queue.
                at = at_sb.tile([128, 512], BF16, tag="at", name=f"at_{g}")
                pvc0 = pvc_sb.tile([65, 512], F32, tag="pvc", name=f"pvc0_{g}")
                pvc1 = pvc_sb.tile([65, 512], F32, tag="pvc", name=f"pvc1_{g}")
                nc.vector.tensor_copy(pvc0[:], pv0[:])
                nc.vector.tensor_copy(pvc1[:], pv1[:])
                nc.gpsimd.dma_start(out=den_dram[2 * g], in_=pvc0[64:65, :])
                nc.gpsimd.dma_start(out=den_dram[2 * g + 1], in_=pvc1[64:65, :])
                dent = den_sb.tile([128, 8], F32, tag="dent", name=f"dent_{g}")
                nc.gpsimd.dma_start(
                    out=dent[:].rearrange("p (a b) -> p a b", a=2),
                    in_=den_dram[2 * g:2 * g + 2, :].rearrange("a (p b) -> p a b", p=128))
                nc.vector.reciprocal(dent[:], dent[:])
                nc.gpsimd.dma_start(
                    out=denr_dram[2 * g:2 * g + 2, :].rearrange("a (p b) -> p a b", p=128),
                    in_=dent[:].rearrange("p (a b) -> p a b", a=2))
                db0 = pvc_sb.tile([64, 512], F32, tag="dbc", name=f"db0_{g}")
                db1 = pvc_sb.tile([64, 512], F32, tag="dbc", name=f"db1_{g}")
                nc.gpsimd.dma_start(
                    out=db0[:], in_=denr_dram[2 * g:2 * g + 1, :].to_broadcast([64, 512]))
                nc.gpsimd.dma_start(
                    out=db1[:], in_=denr_dram[2 * g + 1:2 * g + 2, :].to_broadcast([64, 512]))
                nc.vector.tensor_tensor(
                    at[0:64, :], pvc0[0:64, :], db0[:], mybir.AluOpType.mult)
                nc.vector.tensor_tensor(
                    at[64:128, :], pvc1[0:64, :], db1[:], mybir.AluOpType.mult)
                pending_oproj.append((g, at))

            # ---------------- schedule: projection/attention ladder --------
            proj_strip(0)
            attn_strip(0, 0)
            load_xt_quarter(1)
            proj_strip(1)
            attn_strip(0, 1)
            load_xt_quarter(2)
            proj_strip(2)
            attn_strip(0, 2)
            load_xt_quarter(3)
            proj_strip(3)
            attn_strip(0, 3)
            proj_strip(4)
            proj_strip(5)
            attn_strip(1, 1)
            proj_strip(6)
            attn_strip(1, 2)
            proj_strip(7)
            attn_strip(1, 3)
            attn_strip(1, 0)
            while pending_oproj:
                emit_oproj(*pending_oproj.pop(0))

    nc.finalize()
    return nc


_NC = None


def _get_nc():
    global _NC
    if _NC is None:
        _NC = _build()
    return _NC


def _make_in_maps(x, Wq, bq, Wk, bk, Wv, bv, Wo, bo):
    xT = np.ascontiguousarray(x.reshape(ROWS, D).T).astype(BF16_NP)
    in_maps = []
    for c in range(NCORES):
        sl = slice(c * HD, (c + 1) * HD)
        in_maps.append({
            "xT": xT,
            "wq": np.ascontiguousarray(Wq[:, sl]).astype(BF16_NP),
            "wk": np.ascontiguousarray(Wk[:, sl]).astype(BF16_NP),
            "wv": np.ascontiguousarray(Wv[:, sl]).astype(BF16_NP),
            "bq": np.ascontiguousarray(bq[sl]).reshape(HD, 1).astype(np.float32),
            "bk": np.ascontiguousarray(bk[sl]).reshape(HD, 1).astype(np.float32),
            "bv": np.ascontiguousarray(bv[sl]).reshape(1, HD).astype(np.float32),
            "wo": np.ascontiguousarray(Wo[sl, :]).astype(BF16_NP),
        })
    return in_maps


def _run(inputs, trace=False):
    nc = _get_nc()
    ins = {k: np.asarray(v) for k, v in inputs.items()}
    in_maps = _make_in_maps(**ins)
    res = run_bass_kernel_spmd(nc, in_maps, core_ids=list(range(NCORES)), trace=trace)
    acc = np.zeros((ROWS, D), dtype=np.float32)
    for c in range(NCORES):
        acc += res.results[c]["out"].astype(np.float32)
    acc += ins["bo"].astype(np.float32)
    return acc.reshape(B, S, D), res


def kernel(**inputs):
    out, _ = _run(inputs, trace=False)
    return out


# revision 37
# speedup vs baseline: 1.3489x; 1.0775x over previous
"""Causal multi-head attention block on 8 Trainium2 NeuronCores.

Problem: x:[2,2048,1024] f32 -> MHA(H=16 heads, dk=dv=64, causal) -> [2,2048,1024].

Distribution (tensor-parallel heads, row-parallel output projection):
  - Each core c owns heads {2c, 2c+1}: it gets the matching 128-column slices
    of Wq/Wk/Wv and the matching 128-ROW slice of Wo.
  - Each core computes causal attention for its two heads over all 4096
    (batch*seq) rows, then the row-parallel partial out_c = A_c @ Wo_c for
    ALL rows.  The host sums the 8 partials (the unshard for row-parallel
    TP) and adds bo.  No device collective at all.

Compute dtype bf16 (fp32 PSUM accumulation).  Host supplies x^T pre-cast to
bf16.  Softmax skips the running-max (logits ~N(0,1); exp cannot overflow)
and gets its denominator for free from ones-columns appended to V.

Schedule: a projection/attention ladder — proj(strip g) then attention
(strip g) — so attention (and with it ScalarE's exp stream) starts ~15us in
rather than after all projections.  Each strip's output projection is
emitted two k-tiles into the NEXT strip so its normalize chain (reciprocal
spread over 128 partitions via a DRAM bounce, broadcast back over the idle
gpsimd DMA queue) has landed by the time the PE reaches it.  Everything
except exp runs off ScalarE; PSUM evacuations are split between ScalarE
and VectorE.
"""

import numpy as np
import ml_dtypes

import concourse.mybir as mybir
from concourse import bacc
from concourse.bass_utils import run_bass_kernel_spmd
from concourse.tile import TileContext

F32 = mybir.dt.float32
BF16 = mybir.dt.bfloat16
BF16_NP = ml_dtypes.bfloat16

B, S, D = 2, 2048, 1024
H, DK, DV = 16, 64, 64
ROWS = B * S                  # 4096
NCORES = 8
HPC = H // NCORES             # 2 heads per core
HD = HPC * DK                 # 128 per-core head dim
NSTRIP = ROWS // 512          # 8 global 512-row strips
SCALE = 1.0 / np.sqrt(DK)


def _build():
    nc = bacc.Bacc(None, target_bir_lowering=False, debug=False)

    xT = nc.declare_dram_parameter("xT", [D, ROWS], BF16, isOutput=False)
    wq = nc.declare_dram_parameter("wq", [D, HD], BF16, isOutput=False)
    wk = nc.declare_dram_parameter("wk", [D, HD], BF16, isOutput=False)
    wv = nc.declare_dram_parameter("wv", [D, HD], BF16, isOutput=False)
    bq = nc.declare_dram_parameter("bq", [HD, 1], F32, isOutput=False)
    bk = nc.declare_dram_parameter("bk", [HD, 1], F32, isOutput=False)
    bv = nc.declare_dram_parameter("bv", [1, HD], F32, isOutput=False)
    wo = nc.declare_dram_parameter("wo", [HD, D], BF16, isOutput=False)
    out = nc.declare_dram_parameter("out", [ROWS, D], BF16, isOutput=True)

    with TileContext(nc) as tc:
        with tc.tile_pool(name="const", bufs=1) as csb, \
             tc.tile_pool(name="dram", bufs=1, space="DRAM") as dpool, \
             tc.tile_pool(name="sc_ps", bufs=2, space="PSUM") as sc_ps, \
             tc.tile_pool(name="pv_ps", bufs=2, space="PSUM") as pv_ps, \
             tc.tile_pool(name="mm_ps", bufs=2, space="PSUM") as mm_ps, \
             tc.tile_pool(name="es_sb", bufs=4) as es_sb, \
             tc.tile_pool(name="at_sb", bufs=3) as at_sb, \
             tc.tile_pool(name="den_sb", bufs=6) as den_sb, \
             tc.tile_pool(name="pvc_sb", bufs=6) as pvc_sb, \
             tc.tile_pool(name="osb", bufs=6) as osb_pool:

            # ---------------- constants / weights ----------------
            # triangle keep-mask: mask[kr, q] = 1 if kr <= q else 0
            trimask = csb.tile([128, 128], BF16, name="trimask")
            nc.gpsimd.memset(trimask[:], 1.0)
            nc.gpsimd.affine_select(
                out=trimask[:], in_=trimask[:],
                compare_op=mybir.AluOpType.is_ge, fill=0.0,
                base=0, pattern=[[1, 128]], channel_multiplier=-1,
            )

            wq_sb = csb.tile([128, D], BF16, name="wq_sb")
            wk_sb = csb.tile([128, D], BF16, name="wk_sb")
            wv_sb = csb.tile([128, D], BF16, name="wv_sb")
            wo_sb = csb.tile([128, D], BF16, name="wo_sb")
            bq_sb = csb.tile([HD, 1], F32, name="bq_sb")
            bk_sb = csb.tile([HD, 1], F32, name="bk_sb")
            bv_bc = csb.tile([128, HD], F32, name="bv_bc")
            nc.sync.dma_start(out=wq_sb[:].rearrange("p (a c) -> p a c", a=8), in_=wq[:].rearrange("(a p) c -> p a c", p=128))
            nc.sync.dma_start(out=bq_sb[:], in_=bq[:])
            nc.scalar.dma_start(out=wk_sb[:].rearrange("p (a c) -> p a c", a=8), in_=wk[:].rearrange("(a p) c -> p a c", p=128))
            nc.scalar.dma_start(out=wv_sb[:].rearrange("p (a c) -> p a c", a=8), in_=wv[:].rearrange("(a p) c -> p a c", p=128))
            nc.scalar.dma_start(out=bk_sb[:], in_=bk[:])
            nc.scalar.dma_start(out=bv_bc[:], in_=bv[:].to_broadcast([128, HD]))
            nc.scalar.dma_start(out=wo_sb[:], in_=wo[:])

            # x^T resident in SBUF as one [128, 512] tile per (d-block,
            # strip).  Only strip 0 loads before the Tile entry barrier
            # (which gates on setup-DMA completion); later strips stream in
            # one ladder step ahead, split across both HWDGE trigger queues.
            xt_sb = [[csb.tile([128, 512], BF16, name=f"xt{d}_{g}")
                      for g in range(NSTRIP)] for d in range(8)]

            def load_xt_strip(g):
                for d in range(8):
                    eng = nc.sync if d % 2 == 0 else nc.scalar
                    eng.dma_start(
                        out=xt_sb[d][g][:],
                        in_=xT[d * 128:(d + 1) * 128, g * 512:(g + 1) * 512])

            load_xt_strip(0)

            def xts(d, g, lo, width):
                # x^T slice [128, width] for strip g starting at column lo
                return xt_sb[d][g][:, lo:lo + width]

            # PE clock warm-up while the setup DMAs land
            warm = csb.tile([128, 512], BF16, name="warm")
            nc.gpsimd.memset(warm[:], 0.0)
            wps = mm_ps.tile([128, 512], F32, tag="mm", name="warm_ps")
            for i in range(20):
                nc.tensor.matmul(wps[:], lhsT=warm[:, 0:128], rhs=warm[:],
                                 start=(i == 0), stop=(i == 19))

            den_dram = dpool.tile([2 * NSTRIP, 512], F32, name="den_dram")
            denr_dram = dpool.tile([2 * NSTRIP, 512], F32, name="denr_dram")

            # long-lived per-strip tensors
            qT = [csb.tile([128, 512], BF16, name=f"qT{g}") for g in range(NSTRIP)]
            kTt = [csb.tile([128, 512], BF16, name=f"kT{g}") for g in range(NSTRIP)]
            # v_strip[g]: 4 k-tiles x [v0(64) | one | v1(64) | one] = [128, 520]
            v_strip = [csb.tile([128, 4 * 130], BF16, name=f"v{g}") for g in range(NSTRIP)]

            # ---------------- building blocks ----------------
            ready = set()

            def proj_qk(g, w_sb, b_sb, dst):
                ps = mm_ps.tile([128, 512], F32, tag="mm", name=f"qk_ps_{g}")
                for d in range(8):
                    nc.tensor.matmul(
                        ps[:], lhsT=w_sb[:, d * 128:(d + 1) * 128],
                        rhs=xts(d, g, 0, 512), start=(d == 0), stop=(d == 7))
                nc.vector.tensor_scalar_add(dst[g][:], ps[:], b_sb[:])

            def proj_v(g):
                # V in [rows, dv] layout: 4 row-blocks of [128, 128] in one tile
                ps = mm_ps.tile([128, 512], F32, tag="mm", name=f"v_ps_{g}")
                for rb in range(4):
                    for d in range(8):
                        nc.tensor.matmul(
                            ps[:, rb * 128:(rb + 1) * 128],
                            lhsT=xts(d, g, rb * 128, 128),
                            rhs=wv_sb[:, d * 128:(d + 1) * 128],
                            start=(d == 0), stop=(d == 7))
                nc.gpsimd.memset(v_strip[g][:], 1.0)
                ps3 = ps[:].rearrange("p (k c) -> p k c", k=4)
                v3 = v_strip[g][:].rearrange("p (k c) -> p k c", k=4)
                bv3_0 = bv_bc[:, 0:64].unsqueeze(1).to_broadcast([128, 4, 64])
                bv3_1 = bv_bc[:, 64:128].unsqueeze(1).to_broadcast([128, 4, 64])
                nc.vector.tensor_tensor(
                    v3[:, :, 0:64], ps3[:, :, 0:64], bv3_0, mybir.AluOpType.add)
                nc.vector.tensor_tensor(
                    v3[:, :, 65:129], ps3[:, :, 64:128], bv3_1, mybir.AluOpType.add)

            def proj_strip(g):
                proj_qk(g, wq_sb, bq_sb, qT)
                proj_qk(g, wk_sb, bk_sb, kTt)
                proj_v(g)
                ready.add(g)

            pending_oproj = []

            def emit_oproj(g, at):
                # out[g*512 + qb*128 .. , :] = at[:, qb-block].T @ Wo_c
                for qb in range(4):
                    ot = osb_pool.tile([128, D], BF16, tag="ot", name=f"ot_{g}_{qb}")
                    for n in range(2):
                        ps = mm_ps.tile([128, 512], F32, tag="mm", name=f"o_ps_{g}_{qb}_{n}")
                        nc.tensor.matmul(
                            ps[:], lhsT=at[:, qb * 128:(qb + 1) * 128],
                            rhs=wo_sb[:, n * 512:(n + 1) * 512],
                            start=True, stop=True)
                        # split the PSUM->SBUF evacuations across ACT and DVE
                        if n == 0:
                            nc.scalar.copy(ot[:, n * 512:(n + 1) * 512], ps[:])
                        else:
                            nc.vector.tensor_copy(ot[:, n * 512:(n + 1) * 512], ps[:])
                    nc.sync.dma_start(
                        out=out[g * 512 + qb * 128: g * 512 + (qb + 1) * 128, :],
                        in_=ot[:])

            def attn_strip(b, s):
                g = b * 4 + s
                njt = 4 * (s + 1)
                assert g in ready, f"projections for strip {g} not emitted"
                pv0 = pv_ps.tile([65, 512], F32, tag="pv", name=f"pv0_{g}")
                pv1 = pv_ps.tile([65, 512], F32, tag="pv", name=f"pv1_{g}")
                for j in range(njt):
                    gk = b * 4 + j // 4
                    jj = j % 4
                    qlo = max(0, j - 4 * s) * 128
                    assert gk in ready, f"v/k[{gk}] not emitted (strip {g} j={j})"
                    sc = sc_ps.tile([128, 1024], F32, tag="sc", name=f"sc_{g}_{j}")
                    nc.tensor.matmul(
                        sc[:, qlo:512],
                        lhsT=kTt[gk][0:64, jj * 128:(jj + 1) * 128],
                        rhs=qT[g][0:64, qlo:512], start=True, stop=True)
                    nc.tensor.matmul(
                        sc[:, 512 + qlo:1024],
                        lhsT=kTt[gk][64:128, jj * 128:(jj + 1) * 128],
                        rhs=qT[g][64:128, qlo:512], start=True, stop=True)
                    es = es_sb.tile([128, 1024], BF16, tag="es", name=f"es_{g}_{j}")
                    nc.scalar.activation(
                        es[:, qlo:1024], sc[:, qlo:1024],
                        mybir.ActivationFunctionType.Exp, scale=SCALE)
                    if j >= 4 * s:  # diagonal k-tile: zero kr > q inside the block
                        es3 = es[:].rearrange("p (h w) -> p h w", h=2)[:, :, qlo:qlo + 128]
                        m3 = trimask[:].unsqueeze(1).to_broadcast([128, 2, 128])
                        nc.vector.tensor_tensor(es3, es3, m3, mybir.AluOpType.mult)
                    vb = v_strip[gk][:, jj * 130: (jj + 1) * 130]
                    nc.tensor.matmul(
                        pv0[:, qlo:512], lhsT=vb[:, 0:65],
                        rhs=es[:, qlo:512],
                        start=(j == 0), stop=(j == njt - 1))
                    nc.tensor.matmul(
                        pv1[:, qlo:512], lhsT=vb[:, 65:130],
                        rhs=es[:, 512 + qlo:1024],
                        start=(j == 0), stop=(j == njt - 1))
                    # previous strip's out-projection drops in once its
                    # normalize chain has had time to land
                    if j == min(5, njt - 1) and pending_oproj:
                        emit_oproj(*pending_oproj.pop(0))
                # epilogue: evacuate PV so the pv banks free up immediately,
                # then normalize.  The reciprocal of the 2x512 denominators is
                # spread over 128 partitions via a DRAM round-trip ([1,512]
                # on one DVE lane costs ~3.3us; [128,8] costs ~70ns); all DMA
                # legs ride the otherwise-idle gpsimd queue.
                at = at_sb.tile([128, 512], BF16, tag="at", name=f"at_{g}")
                pvc = pvc_sb.tile([128, 1024], F32, tag="pvc", name=f"pvc_{g}")
                nc.vector.tensor_copy(pvc[0:65, 0:512], pv0[:])
                nc.vector.tensor_copy(pvc[0:65, 512:1024], pv1[:])
                nc.sync.dma_start(
                    out=den_dram[2 * g:2 * g + 2, :].rearrange("a b -> (a b)").unsqueeze(0),
                    in_=pvc[64:65, 0:1024])
                dent = den_sb.tile([128, 8], F32, tag="dent", name=f"dent_{g}")
                nc.gpsimd.dma_start(
                    out=dent[:].rearrange("p (a b) -> p a b", a=2),
                    in_=den_dram[2 * g:2 * g + 2, :].rearrange("a (p b) -> p a b", p=128))
                nc.vector.reciprocal(dent[:], dent[:])
                nc.sync.dma_start(
                    out=denr_dram[2 * g:2 * g + 2, :].rearrange("a (p b) -> p a b", p=128),
                    in_=dent[:].rearrange("p (a b) -> p a b", a=2))
                db = pvc_sb.tile([64, 1024], F32, tag="dbc", name=f"db_{g}")
                nc.gpsimd.dma_start(
                    out=db[:],
                    in_=denr_dram[2 * g:2 * g + 2, :].rearrange("a b -> (a b)")
                        .unsqueeze(0).to_broadcast([64, 1024]))
                nc.vector.tensor_tensor(
                    at[0:64, :], pvc[0:64, 0:512], db[:, 0:512], mybir.AluOpType.mult)
                nc.vector.tensor_tensor(
                    at[64:128, :], pvc[0:64, 512:1024], db[:, 512:1024], mybir.AluOpType.mult)
                pending_oproj.append((g, at))

            # ---------------- schedule: projection/attention ladder --------
            load_xt_strip(1)
            proj_strip(0)
            attn_strip(0, 0)
            load_xt_strip(2)
            proj_strip(1)
            attn_strip(0, 1)
            load_xt_strip(3)
            proj_strip(2)
            attn_strip(0, 2)
            load_xt_strip(4)
            proj_strip(3)
            attn_strip(0, 3)
            load_xt_strip(5)
            proj_strip(4)
            load_xt_strip(6)
            proj_strip(5)
            attn_strip(1, 1)
            load_xt_strip(7)
            proj_strip(6)
            attn_strip(1, 2)
            proj_strip(7)
            attn_strip(1, 3)
            attn_strip(1, 0)
            while pending_oproj:
                emit_oproj(*pending_oproj.pop(0))

    nc.finalize()
    return nc


_NC = None


def _get_nc():
    global _NC
    if _NC is None:
        _NC = _build()
    return _NC


def _make_in_maps(x, Wq, bq, Wk, bk, Wv, bv, Wo, bo):
    xT = np.ascontiguousarray(x.reshape(ROWS, D).T).astype(BF16_NP)
    in_maps = []
    for c in range(NCORES):
        sl = slice(c * HD, (c + 1) * HD)
        in_maps.append({
            "xT": xT,
            "wq": np.ascontiguousarray(Wq[:, sl]).astype(BF16_NP),
            "wk": np.ascontiguousarray(Wk[:, sl]).astype(BF16_NP),
            "wv": np.ascontiguousarray(Wv[:, sl]).astype(BF16_NP),
            "bq": np.ascontiguousarray(bq[sl]).reshape(HD, 1).astype(np.float32),
            "bk": np.ascontiguousarray(bk[sl]).reshape(HD, 1).astype(np.float32),
            "bv": np.ascontiguousarray(bv[sl]).reshape(1, HD).astype(np.float32),
            "wo": np.ascontiguousarray(Wo[sl, :]).astype(BF16_NP),
        })
    return in_maps


def _run(inputs, trace=False):
    nc = _get_nc()
    ins = {k: np.asarray(v) for k, v in inputs.items()}
    in_maps = _make_in_maps(**ins)
    res = run_bass_kernel_spmd(nc, in_maps, core_ids=list(range(NCORES)), trace=trace)
    acc = np.zeros((ROWS, D), dtype=np.float32)
    for c in range(NCORES):
        acc += res.results[c]["out"].astype(np.float32)
    acc += ins["bo"].astype(np.float32)
    return acc.reshape(B, S, D), res


def kernel(**inputs):
    out, _ = _run(inputs, trace=False)
    return out
